# revision 8
# baseline (speedup 1.0000x reference)
"""AttentiveSememe bi-LSTM kernel for Trainium2, 8-core data-parallel.

Model (per batch row):
  w  = word_table[word]                      # [E]
  s  = sem_table[sememes]                    # [L, E]   (row 0 of table is zeros)
  v_g = relu(mean_L(s) @ Wb.T + bb)
  x  = [keep*w ; s]  with keep = (sememes != 0)          # [L, 2E]
  hf/hb = LSTM fwd/bwd over x (PyTorch gate order i,f,g,o)
  V  = relu([hf|hb] @ Wa.T + ba)             # [L, H]

Sharding: batch 1024 -> 8 cores x 128. Each core runs an identical program
on its shard; weights/tables replicated.

Device-side layout: batch (128) on partitions everywhere. All matmuls in
float32r (f32 rounded to 11-bit mantissa; full PE speed at moving dim >= 256).
Gate columns permuted to [i, f, o, g] so sigmoid covers one contiguous slab.
Per step and direction the gate psum [128, 3x512-banks (400 used)] accumulates:
  3 MMs  sT_t chunks   x WsT        (sememe embedding projection)
  1 MM   diag(keep_t)  x u          (u = w @ WwT, precomputed)
  3 MMs  hT chunks+one x WhhT_aug   (recurrent + bias via ones row)
ACT applies sigmoid/tanh, DVE updates c/h, PE transposes h for the next step.
V accumulates into an SBUF stage as each direction's h becomes available.
"""

import os
import sys

sys.path.insert(0, "/opt/trn_rl_repo")

import numpy as np
import concourse.bass as bass
import concourse.tile as tile
from concourse import bacc, mybir
from concourse.bass_utils import run_bass_kernel_spmd

B, L, E, H = 1024, 18, 300, 300
VW, VS = 100000, 2186
NCORES = 8
BC = B // NCORES  # 128
P = 128
EPAD = 320
G4 = 4 * H  # 1200
NSW = [450, 450, 300]  # per-bank gate column widths
NSO = [0, 450, 900]
KSZ = [128, 128, 44]
KOFF = [0, 128, 256]
KA = [128, 128, 45]  # with aug (ones/bias) row on chunk 2

f32 = mybir.dt.float32
f32r = mybir.dt.float32r
i32 = mybir.dt.int32
i16 = mybir.dt.int16

LAST_RESULT = None


def _round_fp32r(a):
    u = np.ascontiguousarray(a, dtype=np.float32).view(np.uint32)
    u2 = (u.astype(np.uint64) + 0x7FF + ((u >> 12) & 1)) & 0xFFFFF000
    return u2.astype(np.uint32).view(np.float32)


def _build():
    nc = bacc.Bacc("TRN2", target_bir_lowering=False, debug=False, enable_asserts=False)
    Sig = mybir.ActivationFunctionType.Sigmoid
    Tanh = mybir.ActivationFunctionType.Tanh
    Relu = mybir.ActivationFunctionType.Relu

    # ---------------- DRAM I/O ----------------
    wtab = nc.dram_tensor("wtab", [VW, E], f32, kind="ExternalInput")
    stab = nc.dram_tensor("stab", [VS, E], f32, kind="ExternalInput")
    widx = nc.dram_tensor("widx", [BC, 1], i32, kind="ExternalInput")
    semi = nc.dram_tensor("semi", [BC, L], i32, kind="ExternalInput")
    ident_d = nc.dram_tensor("ident", [P, P], f32, kind="ExternalInput")
    ones_d = nc.dram_tensor("ones", [1, 2 * P], f32r, kind="ExternalInput")

    ws_d = {}
    ww_d = {}
    whh_d = {}
    wa_d = {}
    wb_d = {}
    for d in ("f", "b"):
        for i in range(3):
            ws_d[d, i] = nc.dram_tensor(f"ws_{d}{i}", [KSZ[i], G4], f32r, kind="ExternalInput")
            ww_d[d, i] = nc.dram_tensor(f"ww_{d}{i}", [KSZ[i], G4], f32r, kind="ExternalInput")
            whh_d[d, i] = nc.dram_tensor(f"whh_{d}{i}", [KA[i], G4], f32r, kind="ExternalInput")
    for i in range(3):
        wa_d["f", i] = nc.dram_tensor(f"wa_f{i}", [KA[i], H], f32r, kind="ExternalInput")
        wa_d["b", i] = nc.dram_tensor(f"wa_b{i}", [KSZ[i], H], f32r, kind="ExternalInput")
        wb_d[i] = nc.dram_tensor(f"wb_{i}", [KA[i], H], f32r, kind="ExternalInput")

    v_out = nc.dram_tensor("v_out", [BC, L * H], f32, kind="ExternalOutput")
    vg_out = nc.dram_tensor("vg_out", [BC, H], f32, kind="ExternalOutput")

    with tile.TileContext(nc) as tc:
        with (
            tc.tile_pool(name="cw", bufs=1) as cw,  # persistent constants/weights
            tc.tile_pool(name="wk", bufs=1) as wk,  # big working tiles
            tc.tile_pool(name="st", bufs=2) as st,  # per-step small tiles
            tc.tile_pool(name="ps", bufs=1, space="PSUM") as ps,
            tc.tile_pool(name="tp", bufs=2, space="PSUM") as tp,
        ):
            # ------------- load constants -------------
            ident = cw.tile([P, P], f32)
            nc.sync.dma_start(ident[:], ident_d[:])
            widx_sb = cw.tile([BC, 1], i32)
            nc.sync.dma_start(widx_sb[:], widx[:])
            semi_sb = cw.tile([BC, L], i32)
            nc.sync.dma_start(semi_sb[:], semi[:])

            # gathers first: they feed the transpose pipeline
            w_sb = wk.tile([BC, E], f32)
            nc.gpsimd.indirect_dma_start(
                out=w_sb[:],
                out_offset=None,
                in_=wtab[:],
                in_offset=bass.IndirectOffsetOnAxis(ap=widx_sb[:, :1], axis=0),
            )
            s_sb = wk.tile([P, L, E], f32, tag="big")
            ORDER = [v for k in range(L // 2) for v in (k, L - 1 - k)]
            for t in ORDER:
                nc.gpsimd.indirect_dma_start(
                    out=s_sb[:, t, :],
                    out_offset=None,
                    in_=stab[:],
                    in_offset=bass.IndirectOffsetOnAxis(ap=semi_sb[:, t : t + 1], axis=0),
                )

            ws = {}
            whh = {}
            for d in ("f", "b"):
                for i in range(3):
                    ws[d, i] = cw.tile([KSZ[i], G4], f32r, name=f"ws{d}{i}")
                    nc.sync.dma_start(ws[d, i][:], ws_d[d, i][:])
                    whh[d, i] = cw.tile([KA[i], G4], f32r, name=f"whh{d}{i}")
                    nc.sync.dma_start(whh[d, i][:], whh_d[d, i][:])
            # ww shares slots with wa (ww dead after u; wa loaded later)
            ww = {}
            for d in ("f", "b"):
                for i in range(3):
                    ww[d, i] = cw.tile([KSZ[i], G4], f32r, name=f"proj{d}{i}", tag=f"proj{d}{i}")
                    nc.sync.dma_start(ww[d, i][:], ww_d[d, i][:])
            wb = {}
            for i in range(3):
                wb[i] = cw.tile([KA[i], H], f32r, name=f"wb{i}")
                nc.sync.dma_start(wb[i][:], wb_d[i][:])

            # ------------- keep mask -------------
            keep = wk.tile([BC, L], f32)
            nc.vector.tensor_scalar(
                out=keep[:], in0=semi_sb[:], scalar1=0, scalar2=None,
                op0=mybir.AluOpType.not_equal,
            )

            # ------------- transposes: w -> wT -------------
            wT = []
            for i in range(3):
                t_ps = tp.tile([P, 128], f32, name="tp", tag="tp")
                nc.tensor.transpose(t_ps[: KSZ[i], :P], w_sb[:, KOFF[i] : KOFF[i] + KSZ[i]], ident[:])
                wt = wk.tile([128, P], f32r, name=f"wT{i}")
                nc.vector.tensor_copy(wt[: KSZ[i], :], t_ps[: KSZ[i], :P])
                wT.append(wt)

            # ------------- u = w @ WwT (both dirs) -------------
            u = {}
            for d in ("f", "b"):
                u_ps = ps.tile([P, 3, 512], f32, name=f"ups{d}", tag=f"g{d}")
                for j in range(3):
                    for i in range(3):
                        nc.tensor.matmul(
                            u_ps[:, j, : NSW[j]],
                            wT[i][: KSZ[i], :],
                            ww[d, i][:, NSO[j] : NSO[j] + NSW[j]],
                            start=(i == 0),
                            stop=(i == 2),
                        )
                ut = wk.tile([BC, G4], f32r, name=f"u{d}")
                nc.vector.tensor_copy(
                    ut[:, 0:900].rearrange("p (j n) -> p j n", j=2), u_ps[:, 0:2, :450]
                )
                nc.vector.tensor_copy(ut[:, 900:1200], u_ps[:, 2, :300])
                u[d] = ut

            # ------------- transposes: s -> sT -------------
            sT = []
            for i in range(3):
                stile = wk.tile([KSZ[i], L * P], f32r, name=f"sT{i}")
                sT.append(stile)
            for t in ORDER:
                for i in range(3):
                    t_ps = tp.tile([P, 128], f32, name="tp", tag="tp")
                    nc.tensor.transpose(
                        t_ps[: KSZ[i], :P],
                        s_sb[:, t, KOFF[i] : KOFF[i] + KSZ[i]],
                        ident[:],
                    )
                    if (t * 3 + i) % 2 == 0:
                        nc.vector.tensor_copy(
                            sT[i][: KSZ[i], t * P : (t + 1) * P], t_ps[: KSZ[i], :P]
                        )
                    else:
                        nc.scalar.copy(
                            sT[i][: KSZ[i], t * P : (t + 1) * P], t_ps[: KSZ[i], :P]
                        )

            # ------------- v_g -------------
            agT = []
            for i in range(3):
                ag = wk.tile([KA[i], P], f32r, name=f"agT{i}")
                agf = wk.tile([KSZ[i], P], f32, name=f"agf{i}")
                nc.vector.reduce_sum(
                    agf[: KSZ[i], :],
                    sT[i][: KSZ[i], :].bitcast(f32).rearrange("k (t p) -> k p t", t=L),
                    axis=mybir.AxisListType.X,
                )
                nc.vector.tensor_copy(ag[: KSZ[i], :], agf[: KSZ[i], :])
                agT.append(ag)
            nc.sync.dma_start(agT[2][44:45, :], ones_d[:, :P])
            vg_ps = ps.tile([P, 3, 512], f32, name="vgps", tag="gf")
            for i in range(3):
                nc.tensor.matmul(
                    vg_ps[:, 0, :H], agT[i][: KA[i], :], wb[i][:], start=(i == 0), stop=(i == 2)
                )
            vg_sb = wk.tile([BC, H], f32)
            nc.scalar.activation(vg_sb[:], vg_ps[:, 0, :H], Relu)
            nc.sync.dma_start(vg_out[:], vg_sb[:])

            # ------------- load wa into proj slots (ww dead) -------------
            wa = {}
            for i in range(3):
                wa["f", i] = cw.tile([KA[i], H], f32r, name=f"waf{i}", tag=f"projf{i}")
                nc.sync.dma_start(wa["f", i][:], wa_d["f", i][:])
                wa["b", i] = cw.tile([KSZ[i], H], f32r, name=f"wab{i}", tag=f"projb{i}")
                nc.sync.dma_start(wa["b", i][:], wa_d["b", i][:])

            # ------------- recurrence state -------------
            hT = {}
            for d in ("f", "b"):
                for i in range(3):
                    ht = wk.tile([KA[i], 2 * P], f32r, name=f"hT{d}{i}")
                    nc.vector.memset(ht[: KSZ[i], 0:P].bitcast(f32), 0.0)  # block 0 = h_init
                    if i == 2:
                        nc.sync.dma_start(ht[44:45, :], ones_d[:, : 2 * P])
                    hT[d, i] = ht
            c = {}
            for d in ("f", "b"):
                ct = wk.tile([BC, H], f32, name=f"c{d}")
                nc.vector.memset(ct[:], 0.0)
                c[d] = ct

            v_stage = wk.tile([P, L * H], f32, tag="big")

            # ------------- recurrence -------------
            for k in range(L):
                for d, pos in (("f", k), ("b", L - 1 - k)):
                    blk = k % 2
                    nblk = (k + 1) % 2
                    # D = diag(keep[:, pos])
                    Dt = st.tile([P, P], f32r, name=f"D{d}")
                    nc.gpsimd.tensor_scalar_mul(Dt[:], ident[:], keep[:, pos : pos + 1])
                    # gate psum
                    g_ps = ps.tile([P, 3, 512], f32, name=f"gps{d}", tag=f"g{d}")
                    for j in range(3):
                        c0, c1 = NSO[j], NSO[j] + NSW[j]
                        for i in range(3):
                            nc.tensor.matmul(
                                g_ps[:, j, : NSW[j]],
                                sT[i][: KSZ[i], pos * P : (pos + 1) * P],
                                ws[d, i][:, c0:c1],
                                start=(i == 0),
                                stop=False,
                            )
                        nc.tensor.matmul(
                            g_ps[:, j, : NSW[j]], Dt[:], u[d][:, c0:c1], start=False, stop=False
                        )
                        for i in range(3):
                            nc.tensor.matmul(
                                g_ps[:, j, : NSW[j]],
                                hT[d, i][: KA[i], blk * P : (blk + 1) * P],
                                whh[d, i][:, c0:c1],
                                start=False,
                                stop=(i == 2),
                            )
                    # activations: layout [i(300) f(300) o(300) | g(300)]
                    gates = st.tile([BC, G4], f32, name=f"gates{d}")
                    nc.scalar.activation(
                        gates[:, 0:900].rearrange("p (j n) -> p j n", j=2),
                        g_ps[:, 0:2, :450],
                        Sig,
                    )
                    nc.scalar.activation(gates[:, 900:1200], g_ps[:, 2, :300], Tanh)
                    # c, h
                    t2 = st.tile([BC, H], f32, name=f"t2{d}")
                    nc.vector.tensor_tensor(
                        out=c[d][:], in0=gates[:, 300:600], in1=c[d][:],
                        op=mybir.AluOpType.mult,
                    )
                    nc.vector.tensor_tensor(
                        out=t2[:], in0=gates[:, 0:300], in1=gates[:, 900:1200],
                        op=mybir.AluOpType.mult,
                    )
                    nc.vector.tensor_tensor(
                        out=c[d][:], in0=c[d][:], in1=t2[:], op=mybir.AluOpType.add
                    )
                    tc_ = st.tile([BC, H], f32, name=f"tc{d}")
                    nc.scalar.activation(tc_[:], c[d][:], Tanh)
                    h = st.tile([BC, H], f32, name=f"h{d}")
                    nc.vector.tensor_tensor(
                        out=h[:], in0=gates[:, 600:900], in1=tc_[:], op=mybir.AluOpType.mult
                    )
                    # h -> hT (next block)
                    for i in range(3):
                        t_ps = tp.tile([P, 128], f32, name="tp", tag="tp")
                        nc.tensor.transpose(
                            t_ps[: KSZ[i], :P], h[:, KOFF[i] : KOFF[i] + KSZ[i]], ident[:]
                        )
                        nc.vector.tensor_copy(
                            hT[d, i][: KSZ[i], nblk * P : (nblk + 1) * P],
                            t_ps[: KSZ[i], :P],
                        )
                    # V half-accumulation, reusing gate psum bank 2 after ACT read it
                    v_ps = g_ps[:, 2, :H]
                    if d == "f":
                        for i in range(3):
                            nc.tensor.matmul(
                                v_ps,
                                hT["f", i][: KA[i], nblk * P : (nblk + 1) * P],
                                wa["f", i][:],
                                start=(i == 0),
                                stop=(i == 2),
                            )
                    else:
                        for i in range(3):
                            nc.tensor.matmul(
                                v_ps,
                                hT["b", i][: KSZ[i], nblk * P : (nblk + 1) * P],
                                wa["b", i][:],
                                start=(i == 0),
                                stop=(i == 2),
                            )
                    vslice = v_stage[:, pos * H : (pos + 1) * H]
                    if k <= 8:
                        nc.scalar.copy(vslice, v_ps)
                    else:
                        nc.vector.tensor_tensor(
                            out=vslice, in0=vslice, in1=v_ps, op=mybir.AluOpType.add
                        )
                        nc.gpsimd.tensor_scalar_max(out=vslice, in0=vslice, scalar1=0.0)
                        nc.sync.dma_start(v_out[:, pos * H : (pos + 1) * H], vslice)

    nc.compile()
    return nc


_NC = None


def _get_nc():
    global _NC
    if _NC is None:
        _NC = _build()
    return _NC


def _prep_inputs(word, sememes, word_table, sem_table,
                 Wih_f, Whh_f, bih_f, bhh_f, Wih_b, Whh_b, bih_b, bhh_b,
                 Wa, ba, Wb, bb):
    word = np.asarray(word).astype(np.int32).reshape(B, 1)
    sememes = np.asarray(sememes).astype(np.int32)
    word_table = np.ascontiguousarray(np.asarray(word_table, dtype=np.float32))
    sem_table = np.asarray(sem_table, dtype=np.float32)

    stab = np.ascontiguousarray(sem_table)

    perm = np.concatenate([np.arange(0, H), np.arange(H, 2 * H),
                           np.arange(3 * H, 4 * H), np.arange(2 * H, 3 * H)])

    shared = {
        "wtab": word_table,
        "stab": stab,
        "ident": np.eye(P, dtype=np.float32),
        "ones": np.ones((1, 2 * P), np.float32),
    }

    for d, Wih, Whh, bih, bhh in (
        ("f", Wih_f, Whh_f, bih_f, bhh_f),
        ("b", Wih_b, Whh_b, bih_b, bhh_b),
    ):
        Wih = np.asarray(Wih, np.float32)
        Whh = np.asarray(Whh, np.float32)
        bsum = (np.asarray(bih, np.float32) + np.asarray(bhh, np.float32))[perm]
        wsT = _round_fp32r(Wih[:, E:].T[:, perm])      # [300, 1200]
        wwT = _round_fp32r(Wih[:, :E].T[:, perm])
        whhT = _round_fp32r(Whh.T[:, perm])            # [300, 1200]
        brow = _round_fp32r(bsum[None, :])
        for i in range(3):
            shared[f"ws_{d}{i}"] = np.ascontiguousarray(wsT[KOFF[i] : KOFF[i] + KSZ[i]])
            shared[f"ww_{d}{i}"] = np.ascontiguousarray(wwT[KOFF[i] : KOFF[i] + KSZ[i]])
            chunk = whhT[KOFF[i] : KOFF[i] + KSZ[i]]
            if i == 2:
                chunk = np.vstack([chunk, brow])
            shared[f"whh_{d}{i}"] = np.ascontiguousarray(chunk)

    WaT = _round_fp32r(np.asarray(Wa, np.float32).T)   # [600, 300]
    ba = _round_fp32r(np.asarray(ba, np.float32)[None, :])
    for i in range(3):
        cf = WaT[KOFF[i] : KOFF[i] + KSZ[i]]
        if i == 2:
            cf = np.vstack([cf, ba])
        shared[f"wa_f{i}"] = np.ascontiguousarray(cf)
        shared[f"wa_b{i}"] = np.ascontiguousarray(WaT[E + KOFF[i] : E + KOFF[i] + KSZ[i]])

    WbT = _round_fp32r(np.asarray(Wb, np.float32).T / L)  # [300, 300] (mean folded)
    bb = _round_fp32r(np.asarray(bb, np.float32)[None, :])
    for i in range(3):
        cb = WbT[KOFF[i] : KOFF[i] + KSZ[i]]
        if i == 2:
            cb = np.vstack([cb, bb])
        shared[f"wb_{i}"] = np.ascontiguousarray(cb)

    in_maps = []
    for cidx in range(NCORES):
        lo = cidx * BC
        sem_c = sememes[lo : lo + BC]  # [128, 18]
        m = dict(shared)
        m["widx"] = np.ascontiguousarray(word[lo : lo + BC])
        m["semi"] = np.ascontiguousarray(sem_c)
        in_maps.append(m)
    return in_maps


def kernel(**inputs):
    global LAST_RESULT
    in_maps = _prep_inputs(**inputs)
    nc = _get_nc()

    if os.environ.get("KERNEL_SIM"):
        from concourse.bass_interp import CoreSim

        ncores = int(os.environ.get("KERNEL_SIM_CORES", "1"))
        results = []
        for cidx in range(ncores):
            sim = CoreSim(nc, trace=False)
            for kk, vv in in_maps[cidx].items():
                sim.tensor(kk)[:] = vv
            sim.simulate()
            results.append({
                "v_out": np.array(sim.tensor("v_out")),
                "vg_out": np.array(sim.tensor("vg_out")),
            })
        while len(results) < NCORES:
            results.append(results[0])
    else:
        res = run_bass_kernel_spmd(nc, in_maps, core_ids=list(range(NCORES)))
        LAST_RESULT = res
        results = res.results

    V = np.concatenate([r["v_out"].reshape(BC, L, H) for r in results], axis=0)
    vg = np.concatenate([r["vg_out"] for r in results], axis=0)
    return V.astype(np.float32), vg.astype(np.float32)


# revision 9
# speedup vs baseline: 1.1524x; 1.1524x over previous
"""AttentiveSememe bi-LSTM kernel for Trainium2, 8-core data-parallel.

Model (per batch row):
  w  = word_table[word]                      # [E]
  s  = sem_table[sememes]                    # [L, E]   (row 0 of table is zeros)
  v_g = relu(mean_L(s) @ Wb.T + bb)
  x  = [keep*w ; s]  with keep = (sememes != 0)          # [L, 2E]
  hf/hb = LSTM fwd/bwd over x (PyTorch gate order i,f,g,o)
  V  = relu([hf|hb] @ Wa.T + ba)             # [L, H]

Sharding: batch 1024 -> 8 cores x 128. Each core runs an identical program
on its shard; weights/tables replicated.

Device-side layout: batch (128) on partitions everywhere. All matmuls in
float32r (f32 rounded to 11-bit mantissa; full PE speed at moving dim >= 256).
Gate columns permuted to [i, f, o, g] so sigmoid covers one contiguous slab.
Per step and direction the gate psum [128, 3x512-banks (400 used)] accumulates:
  3 MMs  sT_t chunks   x WsT        (sememe embedding projection)
  1 MM   diag(keep_t)  x u          (u = w @ WwT, precomputed)
  3 MMs  hT chunks+one x WhhT_aug   (recurrent + bias via ones row)
ACT applies sigmoid/tanh, DVE updates c/h, PE transposes h for the next step.
V accumulates into an SBUF stage as each direction's h becomes available.
"""

import os
import sys

sys.path.insert(0, "/opt/trn_rl_repo")

import numpy as np
import concourse.bass as bass
import concourse.tile as tile
from concourse import bacc, mybir
from concourse.bass_utils import run_bass_kernel_spmd

B, L, E, H = 1024, 18, 300, 300
VW, VS = 100000, 2186
NCORES = 8
BC = B // NCORES  # 128
P = 128
EPAD = 320
G4 = 4 * H  # 1200
NSW = [450, 450, 300]  # per-bank gate column widths
NSO = [0, 450, 900]
KSZ = [128, 128, 44]
KOFF = [0, 128, 256]
KA = [128, 128, 45]  # with aug (ones/bias) row on chunk 2

f32 = mybir.dt.float32
f32r = mybir.dt.float32r
i32 = mybir.dt.int32
i16 = mybir.dt.int16

LAST_RESULT = None


def _round_fp32r(a):
    u = np.ascontiguousarray(a, dtype=np.float32).view(np.uint32)
    u2 = (u.astype(np.uint64) + 0x7FF + ((u >> 12) & 1)) & 0xFFFFF000
    return u2.astype(np.uint32).view(np.float32)


def _build():
    nc = bacc.Bacc("TRN2", target_bir_lowering=False, debug=False, enable_asserts=False)
    Sig = mybir.ActivationFunctionType.Sigmoid
    Tanh = mybir.ActivationFunctionType.Tanh
    Relu = mybir.ActivationFunctionType.Relu

    # ---------------- DRAM I/O ----------------
    wtab = nc.dram_tensor("wtab", [VW, E], f32, kind="ExternalInput")
    stab = nc.dram_tensor("stab", [VS, E], f32, kind="ExternalInput")
    widx = nc.dram_tensor("widx", [BC, 1], i32, kind="ExternalInput")
    semi = nc.dram_tensor("semi", [BC, L], i32, kind="ExternalInput")
    ident_d = nc.dram_tensor("ident", [P, P], f32, kind="ExternalInput")
    ones_d = nc.dram_tensor("ones", [1, 2 * P], f32r, kind="ExternalInput")

    ws_d = {}
    ww_d = {}
    whh_d = {}
    wa_d = {}
    wb_d = {}
    for d in ("f", "b"):
        for i in range(3):
            ws_d[d, i] = nc.dram_tensor(f"ws_{d}{i}", [KSZ[i], G4], f32r, kind="ExternalInput")
            ww_d[d, i] = nc.dram_tensor(f"ww_{d}{i}", [KSZ[i], G4], f32r, kind="ExternalInput")
            whh_d[d, i] = nc.dram_tensor(f"whh_{d}{i}", [KA[i], G4], f32r, kind="ExternalInput")
    for i in range(3):
        wa_d["f", i] = nc.dram_tensor(f"wa_f{i}", [KA[i], H], f32r, kind="ExternalInput")
        wa_d["b", i] = nc.dram_tensor(f"wa_b{i}", [KSZ[i], H], f32r, kind="ExternalInput")
        wb_d[i] = nc.dram_tensor(f"wb_{i}", [KA[i], H], f32r, kind="ExternalInput")

    v_out = nc.dram_tensor("v_out", [BC, L * H], f32, kind="ExternalOutput")
    vg_out = nc.dram_tensor("vg_out", [BC, H], f32, kind="ExternalOutput")

    with tile.TileContext(nc) as tc:
        with (
            tc.tile_pool(name="cw", bufs=1) as cw,  # persistent constants/weights
            tc.tile_pool(name="wk", bufs=1) as wk,  # big working tiles
            tc.tile_pool(name="st", bufs=2) as st,  # per-step small tiles
            tc.tile_pool(name="ps", bufs=1, space="PSUM") as ps,
            tc.tile_pool(name="tp", bufs=2, space="PSUM") as tp,
        ):
            # ------------- load constants -------------
            ident = cw.tile([P, P], f32)
            nc.sync.dma_start(ident[:], ident_d[:])
            widx_sb = cw.tile([BC, 1], i32)
            nc.sync.dma_start(widx_sb[:], widx[:])
            semi_sb = cw.tile([BC, L], i32)
            nc.sync.dma_start(semi_sb[:], semi[:])

            # gathers first: they feed the transpose pipeline
            w_sb = wk.tile([BC, E], f32)
            nc.gpsimd.indirect_dma_start(
                out=w_sb[:],
                out_offset=None,
                in_=wtab[:],
                in_offset=bass.IndirectOffsetOnAxis(ap=widx_sb[:, :1], axis=0),
            )
            s_sb = wk.tile([P, L, E], f32, tag="big")
            ORDER = [v for k in range(L // 2) for v in (k, L - 1 - k)]
            for t in ORDER:
                nc.gpsimd.indirect_dma_start(
                    out=s_sb[:, t, :],
                    out_offset=None,
                    in_=stab[:],
                    in_offset=bass.IndirectOffsetOnAxis(ap=semi_sb[:, t : t + 1], axis=0),
                )

            ws = {}
            whh = {}
            for d in ("f", "b"):
                for i in range(3):
                    ws[d, i] = cw.tile([KSZ[i], G4], f32r, name=f"ws{d}{i}")
                    nc.sync.dma_start(ws[d, i][:], ws_d[d, i][:])
                    whh[d, i] = cw.tile([KA[i], G4], f32r, name=f"whh{d}{i}")
                    nc.sync.dma_start(whh[d, i][:], whh_d[d, i][:])
            # ww shares slots with wa (ww dead after u; wa loaded later)
            ww = {}
            for d in ("f", "b"):
                for i in range(3):
                    ww[d, i] = cw.tile([KSZ[i], G4], f32r, name=f"proj{d}{i}", tag=f"proj{d}{i}")
                    nc.sync.dma_start(ww[d, i][:], ww_d[d, i][:])
            wb = {}
            for i in range(3):
                wb[i] = cw.tile([KA[i], H], f32r, name=f"wb{i}")
                nc.sync.dma_start(wb[i][:], wb_d[i][:])

            # ------------- keep mask -------------
            keep = wk.tile([BC, L], f32)
            nc.vector.tensor_scalar(
                out=keep[:], in0=semi_sb[:], scalar1=0, scalar2=None,
                op0=mybir.AluOpType.not_equal,
            )

            # ------------- transposes: w -> wT -------------
            wT = []
            for i in range(3):
                t_ps = tp.tile([P, 512], f32, name="tp", tag="tp")
                nc.tensor.transpose(t_ps[: KSZ[i], :P], w_sb[:, KOFF[i] : KOFF[i] + KSZ[i]], ident[:])
                wt = wk.tile([128, P], f32r, name=f"wT{i}")
                nc.vector.tensor_copy(wt[: KSZ[i], :], t_ps[: KSZ[i], :P])
                wT.append(wt)

            # ------------- u = w @ WwT (both dirs) -------------
            u = {}
            for d in ("f", "b"):
                u_ps = ps.tile([P, 3, 512], f32, name=f"ups{d}", tag=f"g{d}")
                for j in range(3):
                    for i in range(3):
                        nc.tensor.matmul(
                            u_ps[:, j, : NSW[j]],
                            wT[i][: KSZ[i], :],
                            ww[d, i][:, NSO[j] : NSO[j] + NSW[j]],
                            start=(i == 0),
                            stop=(i == 2),
                        )
                ut = wk.tile([BC, G4], f32r, name=f"u{d}")
                nc.vector.tensor_copy(
                    ut[:, 0:900].rearrange("p (j n) -> p j n", j=2), u_ps[:, 0:2, :450]
                )
                nc.vector.tensor_copy(ut[:, 900:1200], u_ps[:, 2, :300])
                u[d] = ut

            # ------------- transposes: s -> sT -------------
            sT = []
            for i in range(3):
                stile = wk.tile([KSZ[i], L * P], f32r, name=f"sT{i}")
                sT.append(stile)
            for t in ORDER:
                for i in range(3):
                    t_ps = tp.tile([P, 512], f32, name="tp", tag="tp")
                    nc.tensor.transpose(
                        t_ps[: KSZ[i], :P],
                        s_sb[:, t, KOFF[i] : KOFF[i] + KSZ[i]],
                        ident[:],
                    )
                    if (t * 3 + i) % 2 == 0:
                        nc.vector.tensor_copy(
                            sT[i][: KSZ[i], t * P : (t + 1) * P], t_ps[: KSZ[i], :P]
                        )
                    else:
                        nc.scalar.copy(
                            sT[i][: KSZ[i], t * P : (t + 1) * P], t_ps[: KSZ[i], :P]
                        )

            # ------------- v_g -------------
            agT = []
            for i in range(3):
                ag = wk.tile([KA[i], P], f32r, name=f"agT{i}")
                agf = wk.tile([KSZ[i], P], f32, name=f"agf{i}")
                nc.vector.reduce_sum(
                    agf[: KSZ[i], :],
                    sT[i][: KSZ[i], :].bitcast(f32).rearrange("k (t p) -> k p t", t=L),
                    axis=mybir.AxisListType.X,
                )
                nc.vector.tensor_copy(ag[: KSZ[i], :], agf[: KSZ[i], :])
                agT.append(ag)
            nc.sync.dma_start(agT[2][44:45, :], ones_d[:, :P])
            vg_ps = tp.tile([P, 512], f32, name="tp", tag="tp")
            for i in range(3):
                nc.tensor.matmul(
                    vg_ps[:, :H], agT[i][: KA[i], :], wb[i][:], start=(i == 0), stop=(i == 2)
                )
            vg_sb = wk.tile([BC, H], f32)
            nc.scalar.activation(vg_sb[:], vg_ps[:, :H], Relu)
            nc.sync.dma_start(vg_out[:], vg_sb[:])

            # ------------- load wa into proj slots (ww dead) -------------
            wa = {}
            for i in range(3):
                wa["f", i] = cw.tile([KA[i], H], f32r, name=f"waf{i}", tag=f"projf{i}")
                nc.sync.dma_start(wa["f", i][:], wa_d["f", i][:])
                wa["b", i] = cw.tile([KSZ[i], H], f32r, name=f"wab{i}", tag=f"projb{i}")
                nc.sync.dma_start(wa["b", i][:], wa_d["b", i][:])

            # ------------- recurrence state -------------
            hT = {}
            for d in ("f", "b"):
                for i in range(3):
                    ht = wk.tile([KA[i], 2 * P], f32r, name=f"hT{d}{i}")
                    nc.vector.memset(ht[: KSZ[i], 0:P].bitcast(f32), 0.0)  # block 0 = h_init
                    if i == 2:
                        nc.sync.dma_start(ht[44:45, :], ones_d[:, : 2 * P])
                    hT[d, i] = ht
            c = {}
            for d in ("f", "b"):
                ct = wk.tile([BC, H], f32, name=f"c{d}")
                nc.vector.memset(ct[:], 0.0)
                c[d] = ct

            v_stage = wk.tile([P, L * H], f32, tag="big")

            # ------------- recurrence -------------
            for k in range(L):
                for d, pos in (("f", k), ("b", L - 1 - k)):
                    blk = k % 2
                    nblk = (k + 1) % 2
                    # D = diag(keep[:, pos])
                    Dt = st.tile([P, P], f32r, name=f"D{d}")
                    nc.vector.tensor_scalar_mul(Dt[:], ident[:], keep[:, pos : pos + 1])
                    # gate psum
                    g_ps = ps.tile([P, 3, 512], f32, name=f"gps{d}", tag=f"g{d}")
                    for j in range(3):
                        c0, c1 = NSO[j], NSO[j] + NSW[j]
                        for i in range(3):
                            nc.tensor.matmul(
                                g_ps[:, j, : NSW[j]],
                                sT[i][: KSZ[i], pos * P : (pos + 1) * P],
                                ws[d, i][:, c0:c1],
                                start=(i == 0),
                                stop=False,
                            )
                        nc.tensor.matmul(
                            g_ps[:, j, : NSW[j]], Dt[:], u[d][:, c0:c1], start=False, stop=False
                        )
                        for i in range(3):
                            nc.tensor.matmul(
                                g_ps[:, j, : NSW[j]],
                                hT[d, i][: KA[i], blk * P : (blk + 1) * P],
                                whh[d, i][:, c0:c1],
                                start=False,
                                stop=(i == 2),
                            )
                    # activations: layout [i(300) f(300) o(300) | g(300)]
                    gates = st.tile([BC, G4], f32, name=f"gates{d}")
                    nc.scalar.activation(
                        gates[:, 0:900].rearrange("p (j n) -> p j n", j=2),
                        g_ps[:, 0:2, :450],
                        Sig,
                    )
                    nc.scalar.activation(gates[:, 900:1200], g_ps[:, 2, :300], Tanh)
                    # c, h
                    t2 = st.tile([BC, H], f32, name=f"t2{d}")
                    nc.vector.tensor_tensor(
                        out=c[d][:], in0=gates[:, 300:600], in1=c[d][:],
                        op=mybir.AluOpType.mult,
                    )
                    nc.gpsimd.tensor_tensor(
                        out=t2[:], in0=gates[:, 0:300], in1=gates[:, 900:1200],
                        op=mybir.AluOpType.mult,
                    )
                    nc.vector.tensor_tensor(
                        out=c[d][:], in0=c[d][:], in1=t2[:], op=mybir.AluOpType.add
                    )
                    tc_ = st.tile([BC, H], f32, name=f"tc{d}")
                    nc.scalar.activation(tc_[:], c[d][:], Tanh)
                    h = st.tile([BC, H], f32, name=f"h{d}")
                    nc.vector.tensor_tensor(
                        out=h[:], in0=gates[:, 600:900], in1=tc_[:], op=mybir.AluOpType.mult
                    )
                    # h -> hT (next block)
                    for i in range(3):
                        t_ps = tp.tile([P, 512], f32, name="tp", tag="tp")
                        nc.tensor.transpose(
                            t_ps[: KSZ[i], :P], h[:, KOFF[i] : KOFF[i] + KSZ[i]], ident[:]
                        )
                        nc.vector.tensor_copy(
                            hT[d, i][: KSZ[i], nblk * P : (nblk + 1) * P],
                            t_ps[: KSZ[i], :P],
                        )
                    # V half-accumulation for this position
                    v_ps = tp.tile([P, 512], f32, name="tp", tag="tp")
                    if d == "f":
                        for i in range(3):
                            nc.tensor.matmul(
                                v_ps[:, :H],
                                hT["f", i][: KA[i], nblk * P : (nblk + 1) * P],
                                wa["f", i][:],
                                start=(i == 0),
                                stop=(i == 2),
                            )
                    else:
                        for i in range(3):
                            nc.tensor.matmul(
                                v_ps[:, :H],
                                hT["b", i][: KSZ[i], nblk * P : (nblk + 1) * P],
                                wa["b", i][:],
                                start=(i == 0),
                                stop=(i == 2),
                            )
                    vslice = v_stage[:, pos * H : (pos + 1) * H]
                    if k <= 8:
                        nc.scalar.copy(vslice, v_ps[:, :H])
                    else:
                        nc.vector.tensor_tensor(
                            out=vslice, in0=vslice, in1=v_ps[:, :H], op=mybir.AluOpType.add
                        )
                        nc.gpsimd.tensor_scalar_max(out=vslice, in0=vslice, scalar1=0.0)
                        nc.sync.dma_start(v_out[:, pos * H : (pos + 1) * H], vslice)

    nc.compile()
    return nc


_NC = None


def _get_nc():
    global _NC
    if _NC is None:
        _NC = _build()
    return _NC


def _prep_inputs(word, sememes, word_table, sem_table,
                 Wih_f, Whh_f, bih_f, bhh_f, Wih_b, Whh_b, bih_b, bhh_b,
                 Wa, ba, Wb, bb):
    word = np.asarray(word).astype(np.int32).reshape(B, 1)
    sememes = np.asarray(sememes).astype(np.int32)
    word_table = np.ascontiguousarray(np.asarray(word_table, dtype=np.float32))
    sem_table = np.asarray(sem_table, dtype=np.float32)

    stab = np.ascontiguousarray(sem_table)

    perm = np.concatenate([np.arange(0, H), np.arange(H, 2 * H),
                           np.arange(3 * H, 4 * H), np.arange(2 * H, 3 * H)])

    shared = {
        "wtab": word_table,
        "stab": stab,
        "ident": np.eye(P, dtype=np.float32),
        "ones": np.ones((1, 2 * P), np.float32),
    }

    for d, Wih, Whh, bih, bhh in (
        ("f", Wih_f, Whh_f, bih_f, bhh_f),
        ("b", Wih_b, Whh_b, bih_b, bhh_b),
    ):
        Wih = np.asarray(Wih, np.float32)
        Whh = np.asarray(Whh, np.float32)
        bsum = (np.asarray(bih, np.float32) + np.asarray(bhh, np.float32))[perm]
        wsT = _round_fp32r(Wih[:, E:].T[:, perm])      # [300, 1200]
        wwT = _round_fp32r(Wih[:, :E].T[:, perm])
        whhT = _round_fp32r(Whh.T[:, perm])            # [300, 1200]
        brow = _round_fp32r(bsum[None, :])
        for i in range(3):
            shared[f"ws_{d}{i}"] = np.ascontiguousarray(wsT[KOFF[i] : KOFF[i] + KSZ[i]])
            shared[f"ww_{d}{i}"] = np.ascontiguousarray(wwT[KOFF[i] : KOFF[i] + KSZ[i]])
            chunk = whhT[KOFF[i] : KOFF[i] + KSZ[i]]
            if i == 2:
                chunk = np.vstack([chunk, brow])
            shared[f"whh_{d}{i}"] = np.ascontiguousarray(chunk)

    WaT = _round_fp32r(np.asarray(Wa, np.float32).T)   # [600, 300]
    ba = _round_fp32r(np.asarray(ba, np.float32)[None, :])
    for i in range(3):
        cf = WaT[KOFF[i] : KOFF[i] + KSZ[i]]
        if i == 2:
            cf = np.vstack([cf, ba])
        shared[f"wa_f{i}"] = np.ascontiguousarray(cf)
        shared[f"wa_b{i}"] = np.ascontiguousarray(WaT[E + KOFF[i] : E + KOFF[i] + KSZ[i]])

    WbT = _round_fp32r(np.asarray(Wb, np.float32).T / L)  # [300, 300] (mean folded)
    bb = _round_fp32r(np.asarray(bb, np.float32)[None, :])
    for i in range(3):
        cb = WbT[KOFF[i] : KOFF[i] + KSZ[i]]
        if i == 2:
            cb = np.vstack([cb, bb])
        shared[f"wb_{i}"] = np.ascontiguousarray(cb)

    in_maps = []
    for cidx in range(NCORES):
        lo = cidx * BC
        sem_c = sememes[lo : lo + BC]  # [128, 18]
        m = dict(shared)
        m["widx"] = np.ascontiguousarray(word[lo : lo + BC])
        m["semi"] = np.ascontiguousarray(sem_c)
        in_maps.append(m)
    return in_maps


def kernel(**inputs):
    global LAST_RESULT
    in_maps = _prep_inputs(**inputs)
    nc = _get_nc()

    if os.environ.get("KERNEL_SIM"):
        from concourse.bass_interp import CoreSim

        ncores = int(os.environ.get("KERNEL_SIM_CORES", "1"))
        results = []
        for cidx in range(ncores):
            sim = CoreSim(nc, trace=False)
            for kk, vv in in_maps[cidx].items():
                sim.tensor(kk)[:] = vv
            sim.simulate()
            results.append({
                "v_out": np.array(sim.tensor("v_out")),
                "vg_out": np.array(sim.tensor("vg_out")),
            })
        while len(results) < NCORES:
            results.append(results[0])
    else:
        res = run_bass_kernel_spmd(nc, in_maps, core_ids=list(range(NCORES)))
        LAST_RESULT = res
        results = res.results

    V = np.concatenate([r["v_out"].reshape(BC, L, H) for r in results], axis=0)
    vg = np.concatenate([r["vg_out"] for r in results], axis=0)
    return V.astype(np.float32), vg.astype(np.float32)
